# revision 10
# baseline (speedup 1.0000x reference)
"""Trainium2 Bass kernel for nn_DQA_12799002542473 (GNN message passing).

Strategy (8 NeuronCores, zero collectives):
  - Sort edges by destination on host; core c owns target nodes
    [c*32768, (c+1)*32768) and exactly the edges pointing into them.
  - Per core, edges are packed per 128-node range (256 ranges/core) with a
    fixed capacity of 384 slots (3 tiles of 128); pad slots point at row 0
    and carry a -1 local-dst so the one-hot matrix zeroes them.
  - Device: q = x_t@Wq+bq (bf16 table), kv table from layernormed x_source,
    per-edge gather via dma_gather, softmax without segment-max (exp is
    bounded), segment sums via one-hot A matmuls accumulated in PSUM,
    then the gated-update + MLP epilogue, all on-device.
  - The reference's mask bit (sum(layernorm(x_source))!=0) is pure fp
    tie-breaking; it is reproduced bit-exactly on host with jax (index-level
    preprocessing) and encoded in the kv-gather indices (masked edges read
    a [k|0] table row so the denominator still sees k).
"""
import sys
import os

for p in ("/opt/trn_rl_repo", "/root/.axon_site/_ro/trn_rl_repo"):
    if os.path.isdir(p) and p not in sys.path:
        sys.path.insert(0, p)

import numpy as np
import ml_dtypes
from contextlib import ExitStack

import concourse.bass as bass
import concourse.bacc as bacc
import concourse.tile as tile
from concourse import bass_utils, mybir
from concourse._compat import with_exitstack

BF = ml_dtypes.bfloat16
FP = np.float32

N_SRC, N_TGT, N_EDGE = 900, 262144, 500000
D, H, DH = 128, 8, 16
NCORES = 8
SHARD = N_TGT // NCORES        # 32768
RANGES = SHARD // 128          # 256
CAP = 384                      # slots per range
TPR = CAP // 128               # tiles per range = 3
T = RANGES * TPR               # 768 tiles/core
SLOTS = T * 128                # 98304
CH_R = 4                       # ranges per chunk
CH_T = CH_R * TPR              # 12 tiles per chunk
CH_I = CH_T * 128              # 1536 idxs per chunk
NCHUNK = RANGES // CH_R        # 64 chunks
SRC_PAD = 1024                 # padded source rows
KV_ROWS = 2 * SRC_PAD          # rows 1024.. are [k|0] for masked edges
NODE_CH = 64                   # node chunks of 512 for q/E phases

dt = mybir.dt
AF = mybir.ActivationFunctionType
OP = mybir.AluOpType


# ---------------------------------------------------------------- host prep
_MASK_SRC = """
import numpy as np, jax, jax.numpy as jnp, sys
d = np.load(sys.argv[1])
x = jnp.asarray(d["x"]); g = d["g"]; b = d["b"]
mu = jnp.mean(x, axis=-1, keepdims=True)
var = jnp.mean((x - mu) ** 2, axis=-1, keepdims=True)
xs = (x - mu) * jax.lax.rsqrt(var + 1e-5) * g + b
np.save(sys.argv[2], np.asarray(jnp.sum(xs, axis=-1) != 0))
"""


def _mask_plain_jax(x_source, g1, be1):
    import jax
    import jax.numpy as jnp
    x = jnp.asarray(np.asarray(x_source))
    mu = jnp.mean(x, axis=-1, keepdims=True)
    var = jnp.mean((x - mu) ** 2, axis=-1, keepdims=True)
    xs = (x - mu) * jax.lax.rsqrt(var + 1e-5) * np.asarray(g1) + np.asarray(be1)
    return np.asarray(jnp.sum(xs, axis=-1) != 0)


def _reference_mask(x_source, g1, be1):
    """Bit-exact replication of the reference's mask computation (jax cpu,
    plain eager path — the same path the reference harness executes)."""
    try:
        import jax
        if jax.default_backend() == "cpu":
            return _mask_plain_jax(x_source, g1, be1)
    except Exception:
        pass
    # default backend is not cpu (or jax half-broken): replicate in a
    # JAX_PLATFORMS=cpu subprocess
    try:
        import subprocess, tempfile
        with tempfile.TemporaryDirectory() as td:
            fin = os.path.join(td, "in.npz")
            fout = os.path.join(td, "out.npy")
            np.savez(fin, x=np.asarray(x_source, np.float32),
                     g=np.asarray(g1, np.float32), b=np.asarray(be1, np.float32))
            env = dict(os.environ, JAX_PLATFORMS="cpu")
            subprocess.run([sys.executable, "-c", _MASK_SRC, fin, fout],
                           env=env, check=True, capture_output=True)
            return np.load(fout)
    except Exception:
        return np.ones(np.asarray(x_source).shape[0], dtype=bool)


def _wrap_idx(idxs):
    """int16 idxs (len multiple of 128) -> wrapped [128, len/16] layout."""
    n = len(idxs)
    w = np.zeros((128, n // 16), np.int16)
    blk = idxs.reshape(n // 16, 16).T  # [16, n/16]
    for rep in range(8):
        w[rep * 16:(rep + 1) * 16, :] = blk
    return w


def _host_prep(inputs):
    edge_src = np.asarray(inputs["edge_src"])
    edge_dst = np.asarray(inputs["edge_dst"])
    mask = _reference_mask(inputs["x_source"], inputs["g1"], inputs["be1"])

    order = np.argsort(edge_dst, kind="stable")
    s_src = edge_src[order].astype(np.int64)
    s_dst = edge_dst[order].astype(np.int64)

    per_core = []
    for c in range(NCORES):
        lo = np.searchsorted(s_dst, c * SHARD)
        hi = np.searchsorted(s_dst, (c + 1) * SHARD)
        c_src = s_src[lo:hi]
        c_dst = s_dst[lo:hi] - c * SHARD
        rid = c_dst // 128
        rcnt = np.bincount(rid, minlength=RANGES)
        if rcnt.max() > CAP:
            raise RuntimeError(f"range capacity {rcnt.max()} > {CAP}")
        starts = np.zeros(RANGES + 1, np.int64)
        np.cumsum(rcnt, out=starts[1:])

        pad_dst = np.zeros(SLOTS, np.int16)      # q-gather idx (node-local)
        pad_kv = np.zeros(SLOTS, np.int16)       # kv-gather idx
        pad_rloc = np.full(SLOTS, -1.0, FP)      # dst - range_base, -1 = pad
        kv_idx = np.where(mask[c_src], c_src, SRC_PAD + c_src).astype(np.int16)
        for r in range(RANGES):
            n = rcnt[r]
            b = starts[r]
            sl = slice(r * CAP, r * CAP + n)
            pad_dst[sl] = c_dst[b:b + n].astype(np.int16)
            pad_kv[sl] = kv_idx[b:b + n]
            pad_rloc[sl] = (c_dst[b:b + n] - r * 128).astype(FP)

        per_core.append({
            "idx_dst": _wrap_idx(pad_dst),
            "idx_kv": _wrap_idx(pad_kv),
            # [slot_in_tile, tile] layout -> [128, T]
            "rloc": np.ascontiguousarray(pad_rloc.reshape(T, 128).T).astype(BF),
        })
    return per_core, mask


# ------------------------------------------------------------- bass program
@with_exitstack
def _program(ctx: ExitStack, tc: tile.TileContext, io: dict):
    nc = tc.nc
    f32, bf16, i16 = dt.float32, dt.bfloat16, dt.int16

    q_bf = io["q_bf"]
    kv_bf = io["kv_bf"]
    xb_bf = io["xb_bf"]

    cpool = ctx.enter_context(tc.tile_pool(name="consts", bufs=1))

    # ---- weights: load f32, cast to bf16 in SBUF
    w_bf = {}
    for name, cols in [("Wq", D), ("Wk", D), ("Wv", D), ("Wih", D),
                       ("Whh", D), ("Wo", D), ("Wm1", 4 * D)]:
        wf = cpool.tile([128, cols], f32, name=f"{name}f")
        nc.sync.dma_start(wf[:], io[name][:])
        wb = cpool.tile([128, cols], bf16, name=f"{name}b")
        nc.vector.tensor_copy(wb[:], wf[:])
        w_bf[name] = wb
    wm2f = cpool.tile([128, 4, D], f32, name="Wm2f")
    nc.sync.dma_start(wm2f[:], io["Wm2"][:].rearrange("(c p) n -> p c n", p=128))
    wm2b = cpool.tile([128, 4, D], bf16, name="Wm2b")
    nc.vector.tensor_copy(wm2b[:], wm2f[:])

    # ---- small consts
    brows = cpool.tile([1, 16 * 128], bf16, name="brows")
    nc.sync.dma_start(brows[:], io["brows"][:])
    bcols = cpool.tile([128, 8], f32, name="bcols")
    nc.sync.dma_start(bcols[:], io["bcols"][:])
    iota_b = cpool.tile([128, 128], bf16, name="iota_b")
    nc.sync.dma_start(iota_b[:], io["iota_bf"][:])
    id_b = cpool.tile([128, 128], bf16, name="id_b")
    nc.sync.dma_start(id_b[:], io["id_bf"][:])
    e8 = cpool.tile([8, 128], f32, name="e8")
    nc.sync.dma_start(e8[:], io["e8"][:])
    maskc = cpool.tile([128, 8], f32, name="maskc")
    nc.sync.dma_start(maskc[:], io["maskc"][:].rearrange("(a p) o -> p (a o)", p=128))

    ones_row = cpool.tile([1, 128], bf16, name="ones_row")
    nc.vector.memset(ones_row[:], 1.0)
    epscol = cpool.tile([128, 1], f32, name="epscol")
    nc.vector.memset(epscol[:], 1e-16)
    zrow = cpool.tile([128, 128], bf16, name="zrow")
    nc.vector.memset(zrow[:], 0.0)

    # row index in brows
    R_BQ, R_BK, R_BV, R_BO, R_BM2, R_G3, R_BE3, R_G1, R_BE1 = range(9)
    # col index in bcols
    C_BIH, C_BHH, C_BM1 = 0, 1, 2  # bm1 occupies cols 2..5

    bg_col = cpool.tile([128, 1], f32, name="bg_col")
    nc.vector.tensor_add(bg_col[:], bcols[:, C_BIH:C_BIH + 1],
                         bcols[:, C_BHH:C_BHH + 1])

    # replicated rows ([128,128], value varies along free) via K=1 matmul
    with tc.tile_pool(name="init_ps", bufs=1, space="PSUM") as ips:
        reps = {}
        for nm, row in [("g1", R_G1), ("be1", R_BE1), ("g3", R_G3),
                        ("be3", R_BE3)]:
            ps = ips.tile([128, 128], f32, name=f"rep_ps_{nm}", tag="repps")
            nc.tensor.matmul(ps[:], ones_row[:], brows[:, row * 128:(row + 1) * 128],
                             start=True, stop=True)
            rep = cpool.tile([128, 128], bf16, name=f"rep_{nm}")
            nc.vector.tensor_copy(rep[:], ps[:])
            reps[nm] = rep

        # ---- source phase: layernorm(x_source) -> k/v tables in kv_bf
        spool = ctx.enter_context(tc.tile_pool(name="src", bufs=2))
        for t in range(SRC_PAD // 128):
            xsrc = spool.tile([128, 128], f32)
            nc.sync.dma_start(xsrc[:], io["xsrc"][:][t * 128:(t + 1) * 128, :])
            red = spool.tile([128, 1], f32)
            nc.vector.reduce_sum(red[:], xsrc[:], axis=mybir.AxisListType.X)
            muneg = spool.tile([128, 1], f32)
            nc.vector.tensor_scalar(out=muneg[:], in0=red[:],
                                    scalar1=-1.0 / 128, scalar2=None,
                                    op0=OP.mult)
            ss = spool.tile([128, 1], f32)
            sq = spool.tile([128, 128], bf16)
            nc.scalar.activation(out=sq[:], in_=xsrc[:], func=AF.Square,
                                 bias=muneg[:], accum_out=ss[:])
            var = spool.tile([128, 1], f32)
            nc.vector.tensor_scalar(out=var[:], in0=ss[:], scalar1=1.0 / 128,
                                    scalar2=1e-5, op0=OP.mult, op1=OP.add)
            std = spool.tile([128, 1], f32)
            nc.scalar.activation(out=std[:], in_=var[:], func=AF.Sqrt)
            rstd = spool.tile([128, 1], f32)
            nc.vector.reciprocal(rstd[:], std[:])
            xn = spool.tile([128, 128], f32)
            nc.vector.tensor_scalar(out=xn[:], in0=xsrc[:], scalar1=muneg[:],
                                    scalar2=rstd[:], op0=OP.add, op1=OP.mult)
            xg = spool.tile([128, 128], bf16)
            nc.vector.tensor_tensor(out=xg[:], in0=xn[:], in1=reps["g1"][:],
                                    op=OP.mult)
            xs_b = spool.tile([128, 128], bf16)
            nc.vector.tensor_tensor(out=xs_b[:], in0=xg[:], in1=reps["be1"][:],
                                    op=OP.add)
            # transpose -> [feat, src] for matmul lhsT
            tps = ips.tile([128, 128], bf16, name="tps", tag="tps")
            nc.tensor.transpose(tps[:], xs_b[:], id_b[:])
            xs_fm = spool.tile([128, 128], bf16)
            nc.vector.tensor_copy(xs_fm[:], tps[:])

            kps = ips.tile([128, 128], f32, name="kps", tag="kps")
            nc.tensor.matmul(kps[:], xs_fm[:], w_bf["Wk"][:], start=True, stop=False)
            nc.tensor.matmul(kps[:], ones_row[:], brows[:, R_BK * 128:(R_BK + 1) * 128],
                             start=False, stop=True)
            k_sb = spool.tile([128, 128], bf16)
            nc.vector.tensor_copy(k_sb[:], kps[:])
            vps = ips.tile([128, 128], f32, name="vps", tag="kps")
            nc.tensor.matmul(vps[:], xs_fm[:], w_bf["Wv"][:], start=True, stop=False)
            nc.tensor.matmul(vps[:], ones_row[:], brows[:, R_BV * 128:(R_BV + 1) * 128],
                             start=False, stop=True)
            v_sb = spool.tile([128, 128], bf16)
            nc.vector.tensor_scalar(out=v_sb[:], in0=vps[:],
                                    scalar1=maskc[:, t:t + 1], scalar2=None,
                                    op0=OP.mult)
            r0 = t * 128
            nc.sync.dma_start(kv_bf[:][r0:r0 + 128, 0:128], k_sb[:])
            nc.sync.dma_start(kv_bf[:][SRC_PAD + r0:SRC_PAD + r0 + 128, 0:128], k_sb[:])
            nc.sync.dma_start(kv_bf[:][r0:r0 + 128, 128:256], v_sb[:])
            nc.sync.dma_start(kv_bf[:][SRC_PAD + r0:SRC_PAD + r0 + 128, 128:256], zrow[:])

        # ---- q phase: x_t -> xb_bf (cast) and q_bf (projected), 512/chunk
        qpool = ctx.enter_context(tc.tile_pool(name="qp", bufs=3))
        for c in range(int(os.environ.get("KERN_Q_CHUNKS", NODE_CH))):
            n0 = c * 512
            xt_f = qpool.tile([128, 4, 128], f32)
            nc.sync.dma_start(
                xt_f[:], io["xt"][:][n0:n0 + 512, :].rearrange("(a p) d -> p a d", p=128))
            xb_sb = qpool.tile([128, 4, 128], bf16)
            nc.vector.tensor_copy(xb_sb[:], xt_f[:])
            nc.sync.dma_start(
                xb_bf[:][n0:n0 + 512, :].rearrange("(a p) d -> p a d", p=128), xb_sb[:])
            xfmT = qpool.tile([128, 512], bf16)
            nc.sync.dma_start_transpose(xfmT[:], xb_bf[:][n0:n0 + 512, :])
            qps = ips.tile([128, 4, 128], f32, name="qps", tag="qps")
            for b in range(4):
                nc.tensor.matmul(qps[:, b, :], xfmT[:, b * 128:(b + 1) * 128],
                                 w_bf["Wq"][:], start=True, stop=False)
                nc.tensor.matmul(qps[:, b, :], ones_row[:],
                                 brows[:, R_BQ * 128:(R_BQ + 1) * 128], start=False, stop=True)
            q_sb = qpool.tile([128, 4, 128], bf16)
            nc.vector.tensor_copy(q_sb[:], qps[:])
            nc.sync.dma_start(
                q_bf[:][n0:n0 + 512, :].rearrange("(a p) d -> p a d", p=128), q_sb[:])

    # ---- main phase: attention + epilogue, 512 nodes (4 ranges) per chunk
    mp = ctx.enter_context(tc.tile_pool(name="mp", bufs=2))
    pps = ctx.enter_context(tc.tile_pool(name="mps", bufs=1, space="PSUM"))

    n_main = int(os.environ.get("KERN_MAIN_CHUNKS", NCHUNK))
    for c in range(n_main):
        n0 = c * 512
        i0 = c * (CH_I // 16)     # idx col offset
        # -- gathers for 12 edge tiles
        idx_d = mp.tile([128, CH_I // 16], i16, name="idx_d")
        nc.sync.dma_start(idx_d[:], io["idx_dst"][:][:, i0:i0 + CH_I // 16])
        idx_k = mp.tile([128, CH_I // 16], i16, name="idx_k")
        nc.sync.dma_start(idx_k[:], io["idx_kv"][:][:, i0:i0 + CH_I // 16])
        rloc = mp.tile([128, CH_T], bf16, name="rloc")
        nc.sync.dma_start(rloc[:], io["rloc"][:][:, c * CH_T:(c + 1) * CH_T])

        q_g = mp.tile([128, CH_T, 128], bf16, name="q_g")
        nc.gpsimd.dma_gather(out_ap=q_g[:], in_ap=q_bf[:], idxs_ap=idx_d[:],
                             num_idxs=CH_I, num_idxs_reg=CH_I, elem_size=128,
                             single_packet=False)
        kv_g = mp.tile([128, CH_T, 256], bf16, name="kv_g")
        nc.gpsimd.dma_gather(out_ap=kv_g[:], in_ap=kv_bf[:], idxs_ap=idx_k[:],
                             num_idxs=CH_I, num_idxs_reg=CH_I, elem_size=256,
                             single_packet=False)

        # -- per-edge math (batched over the whole chunk)
        t_bf = mp.tile([128, CH_T, 128], bf16, name="t_bf")
        nc.vector.tensor_tensor(out=t_bf[:], in0=q_g[:], in1=kv_g[:, :, 0:128],
                                op=OP.mult)
        alpha = mp.tile([128, CH_T, 8], f32, name="alpha")
        nc.vector.reduce_sum(alpha[:],
                             t_bf[:].rearrange("p t (h d) -> p t h d", h=8),
                             axis=mybir.AxisListType.X)
        u = mp.tile([128, CH_T, 136], bf16, name="u")
        nc.scalar.activation(out=u[:, :, 128:136], in_=alpha[:], func=AF.Exp,
                             scale=0.25)
        nc.vector.tensor_tensor(
            out=u[:, :, 0:128].rearrange("p t (h d) -> p t h d", h=8),
            in0=kv_g[:, :, 128:256].rearrange("p t (h d) -> p t h d", h=8),
            in1=u[:, :, 128:136][:, :, :, None].to_broadcast([128, CH_T, 8, 16]),
            op=OP.mult)
        A = mp.tile([128, CH_T, 128], bf16, name="A")
        nc.vector.tensor_tensor(
            out=A[:],
            in0=iota_b[:][:, None, :].to_broadcast([128, CH_T, 128]),
            in1=rloc[:][:, :, None].to_broadcast([128, CH_T, 128]),
            op=OP.is_equal)

        # -- segment sums into PSUM per range
        num_ps = pps.tile([128, CH_R, 128], f32, name="num_ps", tag="num")
        den_ps = pps.tile([8, CH_R, 128], f32, name="den_ps", tag="den")
        for r in range(CH_R):
            for s in range(TPR):
                t = r * TPR + s
                nc.tensor.matmul(num_ps[:, r, :], u[:, t, 0:128], A[:, t, :],
                                 start=(s == 0), stop=(s == TPR - 1))
                nc.tensor.matmul(den_ps[:, r, :], u[:, t, 128:136], A[:, t, :],
                                 start=(s == 0), stop=(s == TPR - 1))

        # -- agg = num * exp(-ln(den+eps)) broadcast per head
        ln_den = mp.tile([8, CH_R * 128], f32, name="ln_den")
        nc.scalar.activation(out=ln_den[:],
                             in_=den_ps[:].rearrange("p a b -> p (a b)"),
                             func=AF.Ln, bias=epscol[0:8, :])
        lrep_ps = pps.tile([128, 512], f32, name="lrep_ps", tag="lrep")
        nc.tensor.matmul(lrep_ps[:], e8[:], ln_den[:], start=True, stop=True)
        rinv = mp.tile([128, 512], bf16, name="rinv")
        nc.scalar.activation(out=rinv[:], in_=lrep_ps[:], func=AF.Exp,
                             scale=-1.0)
        agg = mp.tile([128, 512], bf16, name="agg")
        nc.vector.tensor_tensor(out=agg[:],
                                in0=num_ps[:].rearrange("p a b -> p (a b)"),
                                in1=rinv[:], op=OP.mult)

        # -- gate + update
        xfmT = mp.tile([128, 512], bf16, name="xfmT2")
        nc.sync.dma_start_transpose(xfmT[:], xb_bf[:][n0:n0 + 512, :])
        gate_ps = pps.tile([128, 512], f32, name="gate_ps", tag="gate")
        nc.tensor.matmul(gate_ps[:], w_bf["Wih"][:], agg[:], start=True, stop=False)
        nc.tensor.matmul(gate_ps[:], w_bf["Whh"][:], xfmT[:], start=False, stop=True)
        gate = mp.tile([128, 512], bf16, name="gate")
        nc.scalar.activation(out=gate[:], in_=gate_ps[:], func=AF.Sigmoid,
                             bias=bg_col[:])
        upd = mp.tile([128, 512], bf16, name="upd")
        nc.vector.tensor_tensor(out=upd[:], in0=agg[:], in1=gate[:], op=OP.mult)

        # -- xt = x + upd@Wo + bo   (node-major)
        xt_ps = pps.tile([128, 4, 128], f32, name="xt_ps", tag="xt")
        for b in range(4):
            nc.tensor.matmul(xt_ps[:, b, :], upd[:, b * 128:(b + 1) * 128],
                             w_bf["Wo"][:], start=True, stop=False)
            nc.tensor.matmul(xt_ps[:, b, :], ones_row[:],
                             brows[:, R_BO * 128:(R_BO + 1) * 128], start=False, stop=True)
        xt_f = mp.tile([128, 4, 128], f32, name="xt_f")
        nc.sync.dma_start(
            xt_f[:], io["xt"][:][n0:n0 + 512, :].rearrange("(a p) d -> p a d", p=128))
        xt_sb = mp.tile([128, 4, 128], f32, name="xt_sb")
        nc.vector.tensor_tensor(out=xt_sb[:], in0=xt_ps[:], in1=xt_f[:], op=OP.add)

        # -- layernorm(xt) node-major
        red = mp.tile([128, 4], f32, name="red")
        nc.vector.reduce_sum(red[:], xt_sb[:], axis=mybir.AxisListType.X)
        muneg = mp.tile([128, 4], f32, name="muneg")
        nc.vector.tensor_scalar(out=muneg[:], in0=red[:], scalar1=-1.0 / 128,
                                scalar2=None, op0=OP.mult)
        ss = mp.tile([128, 4], f32, name="ss")
        sq = mp.tile([128, 128], bf16, name="sq")
        for b in range(4):
            nc.scalar.activation(out=sq[:], in_=xt_sb[:, b, :], func=AF.Square,
                                 bias=muneg[:, b:b + 1], accum_out=ss[:, b:b + 1])
        var = mp.tile([128, 4], f32, name="var")
        nc.vector.tensor_scalar(out=var[:], in0=ss[:], scalar1=1.0 / 128,
                                scalar2=1e-5, op0=OP.mult, op1=OP.add)
        std = mp.tile([128, 4], f32, name="std")
        nc.scalar.activation(out=std[:], in_=var[:], func=AF.Sqrt)
        rstd = mp.tile([128, 4], f32, name="rstd")
        nc.vector.reciprocal(rstd[:], std[:])
        xn = mp.tile([128, 4, 128], f32, name="xn")
        for b in range(4):
            nc.vector.tensor_scalar(out=xn[:, b, :], in0=xt_sb[:, b, :],
                                    scalar1=muneg[:, b:b + 1],
                                    scalar2=rstd[:, b:b + 1],
                                    op0=OP.add, op1=OP.mult)
        hg = mp.tile([128, 4, 128], bf16, name="hg")
        nc.vector.tensor_tensor(
            out=hg[:], in0=xn[:],
            in1=reps["g3"][:][:, None, :].to_broadcast([128, 4, 128]), op=OP.mult)
        h_bf = mp.tile([128, 4, 128], bf16, name="h_bf")
        nc.vector.tensor_tensor(
            out=h_bf[:], in0=hg[:],
            in1=reps["be3"][:][:, None, :].to_broadcast([128, 4, 128]), op=OP.add)

        # -- transpose h -> feature-major
        ht_ps = pps.tile([128, 4, 128], bf16, name="ht_ps", tag="ht")
        for b in range(4):
            nc.tensor.transpose(ht_ps[:, b, :], h_bf[:, b, :], id_b[:])
        h_fm = mp.tile([128, 4, 128], bf16, name="h_fm")
        nc.vector.tensor_copy(h_fm[:], ht_ps[:])
        # h_fm[:, b, :] holds [feat, nodes b*128..] -> view as [128, 512]
        h_fm2 = h_fm[:].rearrange("p a b -> p (a b)")

        # -- mlp
        m1 = []
        for ci in range(4):
            m1_ps = pps.tile([128, 512], f32, name=f"m1_ps{ci}", tag="m1",
                             bufs=2)
            nc.tensor.matmul(m1_ps[:], w_bf["Wm1"][:, ci * 128:(ci + 1) * 128],
                             h_fm2, start=True, stop=True)
            m1_sb = mp.tile([128, 512], bf16, name=f"m1_sb{ci}", tag=f"m1sb{ci}")
            nc.scalar.activation(out=m1_sb[:], in_=m1_ps[:], func=AF.Relu,
                                 bias=bcols[:, C_BM1 + ci:C_BM1 + ci + 1])
            m1.append(m1_sb)
        out_ps = pps.tile([128, 4, 128], f32, name="out_ps", tag="den")
        for b in range(4):
            for ci in range(4):
                nc.tensor.matmul(out_ps[:, b, :],
                                 m1[ci][:, b * 128:(b + 1) * 128],
                                 wm2b[:, ci, :], start=(ci == 0), stop=False)
            nc.tensor.matmul(out_ps[:, b, :], ones_row[:],
                             brows[:, R_BM2 * 128:(R_BM2 + 1) * 128], start=False, stop=True)
        out_sb = mp.tile([128, 4, 128], f32, name="out_sb")
        nc.vector.tensor_tensor(out=out_sb[:], in0=out_ps[:], in1=xt_sb[:],
                                op=OP.add)
        dump = os.environ.get("KERN_DUMP")
        if dump in ("agg", "num", "rinv"):
            src_ap = {"agg": agg[:], "num": num_ps[:].rearrange("p a b -> p (a b)"),
                      "rinv": rinv[:]}[dump]
            agg_f = mp.tile([128, 4, 128], f32, name="agg_f")
            nc.vector.tensor_copy(agg_f[:], src_ap.rearrange("p (a b) -> p a b", a=4))
            nc.sync.dma_start(
                io["out"][:][n0:n0 + 512, :].rearrange("(a p) d -> p a d", p=128),
                agg_f[:])
        else:
            nc.sync.dma_start(
                io["out"][:][n0:n0 + 512, :].rearrange("(a p) d -> p a d", p=128),
                out_sb[:])


# ------------------------------------------------------------------- runner
_CACHE = {}


def _build():
    nc = bacc.Bacc("TRN2", num_devices=NCORES)
    io = {}
    io["xt"] = nc.dram_tensor("xt", [SHARD, D], dt.float32, kind="ExternalInput").ap()
    io["xsrc"] = nc.dram_tensor("xsrc", [SRC_PAD, D], dt.float32, kind="ExternalInput").ap()
    io["maskc"] = nc.dram_tensor("maskc", [SRC_PAD, 1], dt.float32, kind="ExternalInput").ap()
    for name, shape in [("Wq", [D, D]), ("Wk", [D, D]), ("Wv", [D, D]),
                        ("Wih", [D, D]), ("Whh", [D, D]), ("Wo", [D, D]),
                        ("Wm1", [D, 4 * D]), ("Wm2", [4 * D, D])]:
        io[name] = nc.dram_tensor(name, shape, dt.float32, kind="ExternalInput").ap()
    io["brows"] = nc.dram_tensor("brows", [1, 16 * 128], dt.bfloat16, kind="ExternalInput").ap()
    io["bcols"] = nc.dram_tensor("bcols", [128, 8], dt.float32, kind="ExternalInput").ap()
    io["iota_bf"] = nc.dram_tensor("iota_bf", [128, 128], dt.bfloat16, kind="ExternalInput").ap()
    io["id_bf"] = nc.dram_tensor("id_bf", [128, 128], dt.bfloat16, kind="ExternalInput").ap()
    io["e8"] = nc.dram_tensor("e8", [8, 128], dt.float32, kind="ExternalInput").ap()
    io["idx_dst"] = nc.dram_tensor("idx_dst", [128, SLOTS // 16], dt.int16, kind="ExternalInput").ap()
    io["idx_kv"] = nc.dram_tensor("idx_kv", [128, SLOTS // 16], dt.int16, kind="ExternalInput").ap()
    io["rloc"] = nc.dram_tensor("rloc", [128, T], dt.bfloat16, kind="ExternalInput").ap()
    io["q_bf"] = nc.dram_tensor("q_bf", [SHARD, D], dt.bfloat16, kind="Internal").ap()
    io["kv_bf"] = nc.dram_tensor("kv_bf", [KV_ROWS, 256], dt.bfloat16, kind="Internal").ap()
    io["xb_bf"] = nc.dram_tensor("xb_bf", [SHARD, D], dt.bfloat16, kind="Internal").ap()
    io["out"] = nc.dram_tensor("out", [SHARD, D], dt.float32, kind="ExternalOutput").ap()

    with tile.TileContext(nc) as tc:
        _program(tc, io)
    nc.compile()
    return nc


def kernel(**inputs):
    per_core, mask = _host_prep(inputs)

    if "nc" not in _CACHE:
        _CACHE["nc"] = _build()
    nc = _CACHE["nc"]

    xsrc = np.zeros((SRC_PAD, D), FP)
    xsrc[:N_SRC] = np.asarray(inputs["x_source"], FP)
    maskc = np.zeros((SRC_PAD, 1), FP)
    maskc[:N_SRC, 0] = mask.astype(FP)

    brows = np.zeros((1, 16 * 128), BF)
    for i, nm in enumerate(["bq", "bk", "bv", "bo", "bm2", "g3", "be3",
                            "g1", "be1"]):
        brows[0, i * 128:(i + 1) * 128] = np.asarray(inputs[nm]).astype(BF)
    bcols = np.zeros((128, 8), FP)
    bcols[:, 0] = np.asarray(inputs["bih"], FP)
    bcols[:, 1] = np.asarray(inputs["bhh"], FP)
    bcols[:, 2:6] = np.asarray(inputs["bm1"], FP).reshape(4, 128).T
    iota_bf = np.tile(np.arange(128, dtype=FP), (128, 1)).astype(BF)
    id_bf = np.eye(128, dtype=FP).astype(BF)
    e8 = np.zeros((8, 128), FP)
    for h in range(H):
        e8[h, h * DH:(h + 1) * DH] = 1.0

    shared = {
        "xsrc": xsrc, "maskc": maskc, "brows": brows, "bcols": bcols,
        "iota_bf": iota_bf, "id_bf": id_bf, "e8": e8,
    }
    for nm in ["Wq", "Wk", "Wv", "Wih", "Whh", "Wo", "Wm1", "Wm2"]:
        shared[nm] = np.asarray(inputs[nm], FP)

    x_target = np.asarray(inputs["x_target"], FP)
    in_maps = []
    for c in range(NCORES):
        m = dict(shared)
        m["xt"] = np.ascontiguousarray(x_target[c * SHARD:(c + 1) * SHARD])
        m["idx_dst"] = per_core[c]["idx_dst"]
        m["idx_kv"] = per_core[c]["idx_kv"]
        m["rloc"] = per_core[c]["rloc"]
        in_maps.append(m)

    trace = os.environ.get("KERN_TRACE") == "1"
    res = bass_utils.run_bass_kernel_spmd(nc, in_maps, core_ids=list(range(NCORES)),
                                          trace=trace)
    _CACHE["last"] = (nc, in_maps, res)
    out = np.concatenate([r["out"] for r in res.results], axis=0)
    return out.astype(np.float32)


if __name__ == "__main__":
    import importlib.util
    spec = importlib.util.spec_from_file_location("reference", "/root/problem/reference.py")
    ref = importlib.util.module_from_spec(spec)
    spec.loader.exec_module(ref)
    inputs = {k: np.asarray(v) for k, v in ref.setup_inputs().items()}
    out = kernel(**inputs)
    exp = np.asarray(ref.reference(**inputs))
    err = np.abs(out - exp)
    print("absmax:", err.max(), "rel:", err.max() / np.abs(exp).max())


# revision 18
# speedup vs baseline: 21.0091x; 21.0091x over previous
"""Trainium2 Bass kernel for nn_DQA_12799002542473 (GNN message passing).

Strategy (8 NeuronCores, zero collectives):
  - Sort edges by destination on host; core c owns target nodes
    [c*32768, (c+1)*32768) and exactly the edges pointing into them.
  - Per core, edges are packed per 128-node range (256 ranges/core) with a
    fixed capacity of 384 slots (3 tiles of 128); pad slots point at row 0
    and carry a -1 local-dst so the one-hot matrix zeroes them.
  - Device: q = x_t@Wq+bq (bf16 table), kv table from layernormed x_source,
    per-edge gather via dma_gather, softmax without segment-max (exp is
    bounded), segment sums via one-hot A matmuls accumulated in PSUM,
    then the gated-update + MLP epilogue, all on-device.
  - The reference's mask bit (sum(layernorm(x_source))!=0) is pure fp
    tie-breaking; it is reproduced bit-exactly on host with jax (index-level
    preprocessing) and encoded in the kv-gather indices (masked edges read
    a [k|0] table row so the denominator still sees k).
"""
import sys
import os

for p in ("/opt/trn_rl_repo", "/root/.axon_site/_ro/trn_rl_repo"):
    if os.path.isdir(p) and p not in sys.path:
        sys.path.insert(0, p)

import numpy as np
import ml_dtypes
from contextlib import ExitStack

import concourse.bass as bass
import concourse.bacc as bacc
import concourse.tile as tile
from concourse import bass_utils, mybir
from concourse._compat import with_exitstack

BF = ml_dtypes.bfloat16
FP = np.float32

N_SRC, N_TGT, N_EDGE = 900, 262144, 500000
D, H, DH = 128, 8, 16
NCORES = 8
SHARD = N_TGT // NCORES        # 32768
RANGES = SHARD // 128          # 256
CAP = 384                      # slots per range
TPR = CAP // 128               # tiles per range = 3
T = RANGES * TPR               # 768 tiles/core
SLOTS = T * 128                # 98304
CH_R = 4                       # ranges per chunk
CH_T = CH_R * TPR              # 12 tiles per chunk
CH_I = CH_T * 128              # 1536 idxs per chunk
NCHUNK = RANGES // CH_R        # 64 chunks
SRC_PAD = 1024                 # padded source rows
KV_ROWS = 2 * SRC_PAD          # rows 1024.. are [k|0] for masked edges
NODE_CH = 64                   # node chunks of 512 for q/E phases

dt = mybir.dt
AF = mybir.ActivationFunctionType
OP = mybir.AluOpType


# ---------------------------------------------------------------- host prep
_MASK_SRC = """
import numpy as np, jax, jax.numpy as jnp, sys
d = np.load(sys.argv[1])
x = jnp.asarray(d["x"]); g = d["g"]; b = d["b"]
mu = jnp.mean(x, axis=-1, keepdims=True)
var = jnp.mean((x - mu) ** 2, axis=-1, keepdims=True)
xs = (x - mu) * jax.lax.rsqrt(var + 1e-5) * g + b
np.save(sys.argv[2], np.asarray(jnp.sum(xs, axis=-1) != 0))
"""


def _mask_plain_jax(x_source, g1, be1):
    import jax
    import jax.numpy as jnp
    x = jnp.asarray(np.asarray(x_source))
    mu = jnp.mean(x, axis=-1, keepdims=True)
    var = jnp.mean((x - mu) ** 2, axis=-1, keepdims=True)
    xs = (x - mu) * jax.lax.rsqrt(var + 1e-5) * np.asarray(g1) + np.asarray(be1)
    return np.asarray(jnp.sum(xs, axis=-1) != 0)


def _reference_mask(x_source, g1, be1):
    """Bit-exact replication of the reference's mask computation (jax cpu,
    plain eager path — the same path the reference harness executes)."""
    try:
        import jax
        if jax.default_backend() == "cpu":
            return _mask_plain_jax(x_source, g1, be1)
    except Exception:
        pass
    # default backend is not cpu (or jax half-broken): replicate in a
    # JAX_PLATFORMS=cpu subprocess
    try:
        import subprocess, tempfile
        with tempfile.TemporaryDirectory() as td:
            fin = os.path.join(td, "in.npz")
            fout = os.path.join(td, "out.npy")
            np.savez(fin, x=np.asarray(x_source, np.float32),
                     g=np.asarray(g1, np.float32), b=np.asarray(be1, np.float32))
            env = dict(os.environ, JAX_PLATFORMS="cpu")
            subprocess.run([sys.executable, "-c", _MASK_SRC, fin, fout],
                           env=env, check=True, capture_output=True)
            return np.load(fout)
    except Exception:
        return np.ones(np.asarray(x_source).shape[0], dtype=bool)


def _wrap_idx(idxs):
    """int16 idxs (len multiple of 128) -> wrapped [128, len/16] layout."""
    n = len(idxs)
    w = np.zeros((128, n // 16), np.int16)
    blk = idxs.reshape(n // 16, 16).T  # [16, n/16]
    for rep in range(8):
        w[rep * 16:(rep + 1) * 16, :] = blk
    return w


def _host_prep(inputs):
    edge_src = np.asarray(inputs["edge_src"])
    edge_dst = np.asarray(inputs["edge_dst"])
    mask = _reference_mask(inputs["x_source"], inputs["g1"], inputs["be1"])

    order = np.argsort(edge_dst, kind="stable")
    s_src = edge_src[order].astype(np.int64)
    s_dst = edge_dst[order].astype(np.int64)

    per_core = []
    for c in range(NCORES):
        lo = np.searchsorted(s_dst, c * SHARD)
        hi = np.searchsorted(s_dst, (c + 1) * SHARD)
        c_src = s_src[lo:hi]
        c_dst = s_dst[lo:hi] - c * SHARD
        rid = c_dst // 128
        rcnt = np.bincount(rid, minlength=RANGES)
        if rcnt.max() > CAP:
            raise RuntimeError(f"range capacity {rcnt.max()} > {CAP}")
        starts = np.zeros(RANGES + 1, np.int64)
        np.cumsum(rcnt, out=starts[1:])

        pad_dst = np.zeros(SLOTS, np.int16)      # q-gather idx (node-local)
        pad_kv = np.zeros(SLOTS, np.int16)       # kv-gather idx
        pad_rloc = np.full(SLOTS, -1.0, FP)      # dst - range_base, -1 = pad
        kv_idx = np.where(mask[c_src], c_src, SRC_PAD + c_src).astype(np.int16)
        for r in range(RANGES):
            n = rcnt[r]
            b = starts[r]
            sl = slice(r * CAP, r * CAP + n)
            pad_dst[sl] = c_dst[b:b + n].astype(np.int16)
            pad_kv[sl] = kv_idx[b:b + n]
            pad_rloc[sl] = (c_dst[b:b + n] - r * 128).astype(FP)

        per_core.append({
            "idx_dst": _wrap_idx(pad_dst),
            "idx_kv": _wrap_idx(pad_kv),
            # [slot_in_tile, tile] layout -> [128, T]
            "rloc": np.ascontiguousarray(pad_rloc.reshape(T, 128).T).astype(FP),
        })
    return per_core, mask


# ------------------------------------------------------------- bass program
@with_exitstack
def _program(ctx: ExitStack, tc: tile.TileContext, io: dict):
    nc = tc.nc
    f32, bf16, i16 = dt.float32, dt.bfloat16, dt.int16

    q_bf = io["q_bf"]
    kv_bf = io["kv_bf"]
    xb_bf = io["xb_bf"]

    cpool = ctx.enter_context(tc.tile_pool(name="consts", bufs=1))

    # ---- weights: load f32, cast to bf16 in SBUF
    w_bf = {}
    for name, cols in [("Wq", D), ("Wk", D), ("Wv", D), ("Wih", D),
                       ("Whh", D), ("Wo", D), ("Wm1", 4 * D)]:
        wf = cpool.tile([128, cols], f32, name=f"{name}f")
        nc.sync.dma_start(wf[:], io[name][:])
        wb = cpool.tile([128, cols], bf16, name=f"{name}b")
        nc.vector.tensor_copy(wb[:], wf[:])
        w_bf[name] = wb
    wm2f = cpool.tile([128, 4, D], f32, name="Wm2f")
    nc.sync.dma_start(wm2f[:], io["Wm2"][:].rearrange("(c p) n -> p c n", p=128))
    wm2b = cpool.tile([128, 4, D], bf16, name="Wm2b")
    nc.vector.tensor_copy(wm2b[:], wm2f[:])

    # ---- small consts
    brows = cpool.tile([1, 16 * 128], bf16, name="brows")
    nc.sync.dma_start(brows[:], io["brows"][:])
    bcols = cpool.tile([128, 8], f32, name="bcols")
    nc.sync.dma_start(bcols[:], io["bcols"][:])
    iota_b = cpool.tile([128, 128], bf16, name="iota_b")
    nc.sync.dma_start(iota_b[:], io["iota_bf"][:])
    id_b = cpool.tile([128, 128], bf16, name="id_b")
    nc.sync.dma_start(id_b[:], io["id_bf"][:])
    e8 = cpool.tile([8, 128], f32, name="e8")
    nc.sync.dma_start(e8[:], io["e8"][:])
    maskc = cpool.tile([128, 8], f32, name="maskc")
    nc.sync.dma_start(maskc[:], io["maskc"][:].rearrange("(a p) o -> p (a o)", p=128))

    ones_row = cpool.tile([1, 128], bf16, name="ones_row")
    nc.vector.memset(ones_row[:], 1.0)
    epscol = cpool.tile([128, 1], f32, name="epscol")
    nc.vector.memset(epscol[:], 1e-16)
    zrow = cpool.tile([128, 128], bf16, name="zrow")
    nc.vector.memset(zrow[:], 0.0)

    # row index in brows
    R_BQ, R_BK, R_BV, R_BO, R_BM2, R_G3, R_BE3, R_G1, R_BE1 = range(9)
    # col index in bcols
    C_BIH, C_BHH, C_BM1 = 0, 1, 2  # bm1 occupies cols 2..5

    bg_col = cpool.tile([128, 1], f32, name="bg_col")
    nc.vector.tensor_add(bg_col[:], bcols[:, C_BIH:C_BIH + 1],
                         bcols[:, C_BHH:C_BHH + 1])

    # replicated rows ([128,128], value varies along free) via K=1 matmul
    with tc.tile_pool(name="init_ps", bufs=1, space="PSUM") as ips:
        reps = {}
        for nm, row in [("g1", R_G1), ("be1", R_BE1), ("g3", R_G3),
                        ("be3", R_BE3)]:
            ps = ips.tile([128, 128], f32, name=f"rep_ps_{nm}", tag="repps")
            nc.tensor.matmul(ps[:], ones_row[:], brows[:, row * 128:(row + 1) * 128],
                             start=True, stop=True)
            rep = cpool.tile([128, 128], bf16, name=f"rep_{nm}")
            nc.vector.tensor_copy(rep[:], ps[:])
            reps[nm] = rep

        # ---- source phase: layernorm(x_source) -> k/v tables in kv_bf
        spool = ctx.enter_context(tc.tile_pool(name="src", bufs=2))
        for t in range(SRC_PAD // 128):
            xsrc = spool.tile([128, 128], f32)
            nc.sync.dma_start(xsrc[:], io["xsrc"][:][t * 128:(t + 1) * 128, :])
            red = spool.tile([128, 1], f32)
            nc.vector.reduce_sum(red[:], xsrc[:], axis=mybir.AxisListType.X)
            muneg = spool.tile([128, 1], f32)
            nc.vector.tensor_scalar(out=muneg[:], in0=red[:],
                                    scalar1=-1.0 / 128, scalar2=None,
                                    op0=OP.mult)
            ss = spool.tile([128, 1], f32)
            sq = spool.tile([128, 128], bf16)
            nc.scalar.activation(out=sq[:], in_=xsrc[:], func=AF.Square,
                                 bias=muneg[:], accum_out=ss[:])
            var = spool.tile([128, 1], f32)
            nc.vector.tensor_scalar(out=var[:], in0=ss[:], scalar1=1.0 / 128,
                                    scalar2=1e-5, op0=OP.mult, op1=OP.add)
            std = spool.tile([128, 1], f32)
            nc.scalar.activation(out=std[:], in_=var[:], func=AF.Sqrt)
            rstd = spool.tile([128, 1], f32)
            nc.vector.reciprocal(rstd[:], std[:])
            xn = spool.tile([128, 128], f32)
            nc.vector.tensor_scalar(out=xn[:], in0=xsrc[:], scalar1=muneg[:],
                                    scalar2=rstd[:], op0=OP.add, op1=OP.mult)
            xg = spool.tile([128, 128], bf16)
            nc.vector.tensor_tensor(out=xg[:], in0=xn[:], in1=reps["g1"][:],
                                    op=OP.mult)
            xs_b = spool.tile([128, 128], bf16)
            nc.vector.tensor_tensor(out=xs_b[:], in0=xg[:], in1=reps["be1"][:],
                                    op=OP.add)
            # transpose -> [feat, src] for matmul lhsT
            tps = ips.tile([128, 128], bf16, name="tps", tag="tps")
            nc.tensor.transpose(tps[:], xs_b[:], id_b[:])
            xs_fm = spool.tile([128, 128], bf16)
            nc.vector.tensor_copy(xs_fm[:], tps[:])

            kps = ips.tile([128, 128], f32, name="kps", tag="kps")
            nc.tensor.matmul(kps[:], xs_fm[:], w_bf["Wk"][:], start=True, stop=False)
            nc.tensor.matmul(kps[:], ones_row[:], brows[:, R_BK * 128:(R_BK + 1) * 128],
                             start=False, stop=True)
            k_sb = spool.tile([128, 128], bf16)
            nc.vector.tensor_copy(k_sb[:], kps[:])
            vps = ips.tile([128, 128], f32, name="vps", tag="kps")
            nc.tensor.matmul(vps[:], xs_fm[:], w_bf["Wv"][:], start=True, stop=False)
            nc.tensor.matmul(vps[:], ones_row[:], brows[:, R_BV * 128:(R_BV + 1) * 128],
                             start=False, stop=True)
            v_sb = spool.tile([128, 128], bf16)
            nc.vector.tensor_scalar(out=v_sb[:], in0=vps[:],
                                    scalar1=maskc[:, t:t + 1], scalar2=None,
                                    op0=OP.mult)
            r0 = t * 128
            nc.sync.dma_start(kv_bf[:][r0:r0 + 128, 0:128], k_sb[:])
            nc.sync.dma_start(kv_bf[:][SRC_PAD + r0:SRC_PAD + r0 + 128, 0:128], k_sb[:])
            nc.sync.dma_start(kv_bf[:][r0:r0 + 128, 128:256], v_sb[:])
            nc.sync.dma_start(kv_bf[:][SRC_PAD + r0:SRC_PAD + r0 + 128, 128:256], zrow[:])

        # ---- q phase: x_t -> xb_bf (cast) and q_bf (projected), 512/chunk
        qpool = ctx.enter_context(tc.tile_pool(name="qp", bufs=3))
        for c in range(int(os.environ.get("KERN_Q_CHUNKS", NODE_CH))):
            n0 = c * 512
            xt_f = qpool.tile([128, 4, 128], f32)
            nc.sync.dma_start(
                xt_f[:], io["xt"][:][n0:n0 + 512, :].rearrange("(a p) d -> p a d", p=128))
            xb_sb = qpool.tile([128, 4, 128], bf16)
            nc.vector.tensor_copy(xb_sb[:], xt_f[:])
            xfmT = qpool.tile([128, 512], bf16)
            if os.environ.get("KERN_PET", "1") == "1":
                xt_tp = ips.tile([128, 4, 128], bf16, name="xt_tp", tag="xt_tp",
                                 bufs=2)
                for b in range(4):
                    nc.tensor.transpose(xt_tp[:, b, :], xb_sb[:, b, :], id_b[:])
                nc.vector.tensor_copy(
                    xfmT[:].rearrange("p (a b) -> p a b", a=4), xt_tp[:])
            else:
                nc.sync.dma_start(
                    xb_bf[:][n0:n0 + 512, :].rearrange("(a p) d -> p a d", p=128), xb_sb[:])
                nc.sync.dma_start_transpose(xfmT[:], xb_bf[:][n0:n0 + 512, :])
            qps = ips.tile([128, 4, 128], f32, name="qps", tag="qps", bufs=3)
            for b in range(4):
                nc.tensor.matmul(qps[:, b, :], xfmT[:, b * 128:(b + 1) * 128],
                                 w_bf["Wq"][:], start=True, stop=False)
                nc.tensor.matmul(qps[:, b, :], ones_row[:],
                                 brows[:, R_BQ * 128:(R_BQ + 1) * 128], start=False, stop=True)
            q_sb = qpool.tile([128, 4, 128], bf16)
            nc.vector.tensor_copy(q_sb[:], qps[:])
            nc.sync.dma_start(
                q_bf[:][n0:n0 + 512, :].rearrange("(a p) d -> p a d", p=128), q_sb[:])

    NQ = int(os.environ.get("KERN_NQ", "1"))
    # ---- main phase: attention + epilogue, 512 nodes (4 ranges) per chunk
    mp = ctx.enter_context(tc.tile_pool(name="mp", bufs=int(os.environ.get("KERN_MPBUFS", "3"))))
    pps = ctx.enter_context(tc.tile_pool(name="mps", bufs=1, space="PSUM"))

    n_main = int(os.environ.get("KERN_MAIN_CHUNKS", NCHUNK))
    for c in range(n_main):
        n0 = c * 512
        i0 = c * (CH_I // 16)     # idx col offset
        # -- gathers for 12 edge tiles
        idx_d = mp.tile([128, CH_I // 16], i16, name="idx_d")
        nc.sync.dma_start(idx_d[:], io["idx_dst"][:][:, i0:i0 + CH_I // 16])
        idx_k = mp.tile([128, CH_I // 16], i16, name="idx_k")
        nc.sync.dma_start(idx_k[:], io["idx_kv"][:][:, i0:i0 + CH_I // 16])
        rloc = mp.tile([128, CH_T], f32, name="rloc")
        nc.sync.dma_start(rloc[:], io["rloc"][:][:, c * CH_T:(c + 1) * CH_T])

        q_g = mp.tile([128, CH_T, 128], bf16, name="q_g")
        kv_g = mp.tile([128, CH_T, 256], bf16, name="kv_g")
        if os.environ.get("KERN_NO_GATHER") == "1":
            nc.sync.dma_start(q_g[:], q_bf[:][0:1536, :].rearrange("(a p) d -> p a d", p=128))
            nc.sync.dma_start(kv_g[:], kv_bf[:][0:1536, :].rearrange("(a p) d -> p a d", p=128))
        else:
            nc.gpsimd.dma_gather(out_ap=q_g[:], in_ap=q_bf[:], idxs_ap=idx_d[:],
                                 num_idxs=CH_I, num_idxs_reg=CH_I, elem_size=128,
                                 single_packet=False,
                                 queue_num=(c % NQ))
            nc.gpsimd.dma_gather(out_ap=kv_g[:], in_ap=kv_bf[:], idxs_ap=idx_k[:],
                                 num_idxs=CH_I, num_idxs_reg=CH_I, elem_size=256,
                                 single_packet=False,
                                 queue_num=((c + 1) % NQ))

        # -- per-edge math (batched over the whole chunk)
        SKIP = os.environ.get("KERN_SKIP", "")
        t_bf = mp.tile([128, CH_T, 128], bf16, name="t_bf")
        if "edvE" not in SKIP:
            nc.vector.tensor_tensor(out=t_bf[:], in0=q_g[:], in1=kv_g[:, :, 0:128],
                                    op=OP.mult)
        alpha = mp.tile([128, CH_T, 8], f32, name="alpha")
        # (stage-skip scaffolding below)
        nc.vector.reduce_sum(alpha[:],
                             t_bf[:].rearrange("p t (h d) -> p t h d", h=8),
                             axis=mybir.AxisListType.X)
        u = mp.tile([128, CH_T, 136], bf16, name="u")
        nc.scalar.activation(out=u[:, :, 128:136], in_=alpha[:], func=AF.Exp,
                             scale=0.25)
        p_rep = mp.tile([128, CH_T, 8, 16], bf16, name="p_rep")
        nc.scalar.activation(
            out=p_rep[:],
            in_=u[:, :, 128:136][:, :, :, None].to_broadcast([128, CH_T, 8, 16]),
            func=AF.Copy)
        nc.vector.tensor_tensor(
            out=u[:, :, 0:128].rearrange("p t (h d) -> p t h d", h=8),
            in0=kv_g[:, :, 128:256].rearrange("p t (h d) -> p t h d", h=8),
            in1=p_rep[:], op=OP.mult)
        A = mp.tile([128, CH_T, 128], bf16, name="A")
        for t in range(CH_T):
            nc.vector.tensor_scalar(
                out=A[:, t, :], in0=iota_b[:], scalar1=rloc[:, t:t + 1],
                scalar2=None, op0=OP.is_equal)

        # -- segment sums into PSUM per range
        num_ps = pps.tile([128, CH_R, 128], f32, name="num_ps", tag="num", bufs=2)
        den_ps = pps.tile([8, CH_R, 128], f32, name="den_ps", tag="den", bufs=2)
        for r in range(CH_R):
            for s in range(TPR):
                t = r * TPR + s
                nc.tensor.matmul(num_ps[:, r, :], u[:, t, 0:128], A[:, t, :],
                                 start=(s == 0), stop=(s == TPR - 1))
                nc.tensor.matmul(den_ps[:, r, :], u[:, t, 128:136], A[:, t, :],
                                 start=(s == 0), stop=(s == TPR - 1))

        # -- agg = num * exp(-ln(den+eps)) broadcast per head
        ln_den = mp.tile([8, CH_R * 128], f32, name="ln_den")
        nc.scalar.activation(out=ln_den[:],
                             in_=den_ps[:].rearrange("p a b -> p (a b)"),
                             func=AF.Ln, bias=epscol[0:8, :])
        lrep_ps = pps.tile([128, 512], f32, name="lrep_ps", tag="gate")
        nc.tensor.matmul(lrep_ps[:], e8[:], ln_den[:], start=True, stop=True)
        rinv = mp.tile([128, 512], bf16, name="rinv")
        nc.scalar.activation(out=rinv[:], in_=lrep_ps[:], func=AF.Exp,
                             scale=-1.0)
        agg = mp.tile([128, 512], bf16, name="agg")
        nc.vector.tensor_tensor(out=agg[:],
                                in0=num_ps[:].rearrange("p a b -> p (a b)"),
                                in1=rinv[:], op=OP.mult)

        # -- gate + update
        xt_f = mp.tile([128, 4, 128], f32, name="xt_f")
        nc.sync.dma_start(
            xt_f[:], io["xt"][:][n0:n0 + 512, :].rearrange("(a p) d -> p a d", p=128))
        xfmT = mp.tile([128, 512], bf16, name="xfmT2")
        if os.environ.get("KERN_PET", "1") == "1":
            xb2 = mp.tile([128, 4, 128], bf16, name="xb2")
            nc.vector.tensor_copy(xb2[:], xt_f[:])
            xt_tp2 = pps.tile([128, 4, 128], bf16, name="xt_tp2", tag="xt")
            for b in range(4):
                nc.tensor.transpose(xt_tp2[:, b, :], xb2[:, b, :], id_b[:])
            nc.vector.tensor_copy(
                xfmT[:].rearrange("p (a b) -> p a b", a=4), xt_tp2[:])
        else:
            nc.sync.dma_start_transpose(xfmT[:], xb_bf[:][n0:n0 + 512, :])
        gate_ps = pps.tile([128, 512], f32, name="gate_ps", tag="gate")
        nc.tensor.matmul(gate_ps[:], w_bf["Wih"][:], agg[:], start=True, stop=False)
        nc.tensor.matmul(gate_ps[:], w_bf["Whh"][:], xfmT[:], start=False, stop=True)
        gate = mp.tile([128, 512], bf16, name="gate")
        nc.scalar.activation(out=gate[:], in_=gate_ps[:], func=AF.Sigmoid,
                             bias=bg_col[:])
        upd = mp.tile([128, 512], bf16, name="upd")
        nc.vector.tensor_tensor(out=upd[:], in0=agg[:], in1=gate[:], op=OP.mult)

        # -- xt = x + upd@Wo + bo   (node-major)
        xt_ps = pps.tile([128, 4, 128], f32, name="xt_ps", tag="xt")
        for b in range(4):
            nc.tensor.matmul(xt_ps[:, b, :], upd[:, b * 128:(b + 1) * 128],
                             w_bf["Wo"][:], start=True, stop=False)
            nc.tensor.matmul(xt_ps[:, b, :], ones_row[:],
                             brows[:, R_BO * 128:(R_BO + 1) * 128], start=False, stop=True)
        xt_sb = mp.tile([128, 4, 128], f32, name="xt_sb")
        nc.vector.tensor_tensor(out=xt_sb[:], in0=xt_ps[:], in1=xt_f[:], op=OP.add)

        # -- layernorm(xt) node-major
        red = mp.tile([128, 4], f32, name="red")
        nc.vector.reduce_sum(red[:], xt_sb[:], axis=mybir.AxisListType.X)
        muneg = mp.tile([128, 4], f32, name="muneg")
        nc.vector.tensor_scalar(out=muneg[:], in0=red[:], scalar1=-1.0 / 128,
                                scalar2=None, op0=OP.mult)
        ss = mp.tile([128, 4], f32, name="ss")
        sq = mp.tile([128, 128], bf16, name="sq")
        for b in range(4):
            nc.scalar.activation(out=sq[:], in_=xt_sb[:, b, :], func=AF.Square,
                                 bias=muneg[:, b:b + 1], accum_out=ss[:, b:b + 1])
        var = mp.tile([128, 4], f32, name="var")
        nc.vector.tensor_scalar(out=var[:], in0=ss[:], scalar1=1.0 / 128,
                                scalar2=1e-5, op0=OP.mult, op1=OP.add)
        std = mp.tile([128, 4], f32, name="std")
        nc.scalar.activation(out=std[:], in_=var[:], func=AF.Sqrt)
        rstd = mp.tile([128, 4], f32, name="rstd")
        nc.vector.reciprocal(rstd[:], std[:])
        xn = mp.tile([128, 4, 128], f32, name="xn")
        for b in range(4):
            nc.vector.tensor_scalar(out=xn[:, b, :], in0=xt_sb[:, b, :],
                                    scalar1=muneg[:, b:b + 1],
                                    scalar2=rstd[:, b:b + 1],
                                    op0=OP.add, op1=OP.mult)
        hg = mp.tile([128, 4, 128], bf16, name="hg")
        nc.vector.tensor_tensor(
            out=hg[:], in0=xn[:],
            in1=reps["g3"][:][:, None, :].to_broadcast([128, 4, 128]), op=OP.mult)
        h_bf = mp.tile([128, 4, 128], bf16, name="h_bf")
        nc.vector.tensor_tensor(
            out=h_bf[:], in0=hg[:],
            in1=reps["be3"][:][:, None, :].to_broadcast([128, 4, 128]), op=OP.add)

        # -- transpose h -> feature-major
        ht_ps = pps.tile([128, 4, 128], bf16, name="ht_ps", tag="m1", bufs=2)
        for b in range(4):
            nc.tensor.transpose(ht_ps[:, b, :], h_bf[:, b, :], id_b[:])
        h_fm = mp.tile([128, 4, 128], bf16, name="h_fm")
        nc.vector.tensor_copy(h_fm[:], ht_ps[:])
        # h_fm[:, b, :] holds [feat, nodes b*128..] -> view as [128, 512]
        h_fm2 = h_fm[:].rearrange("p a b -> p (a b)")

        # -- mlp
        m1 = []
        for ci in range(4):
            m1_ps = pps.tile([128, 512], f32, name=f"m1_ps{ci}", tag="m1",
                             bufs=2)
            nc.tensor.matmul(m1_ps[:], w_bf["Wm1"][:, ci * 128:(ci + 1) * 128],
                             h_fm2, start=True, stop=True)
            m1_sb = mp.tile([128, 512], bf16, name=f"m1_sb{ci}", tag=f"m1sb{ci}")
            nc.scalar.activation(out=m1_sb[:], in_=m1_ps[:], func=AF.Relu,
                                 bias=bcols[:, C_BM1 + ci:C_BM1 + ci + 1])
            m1.append(m1_sb)
        out_ps = pps.tile([128, 4, 128], f32, name="out_ps", tag="den", bufs=2)
        for b in range(4):
            for ci in range(4):
                nc.tensor.matmul(out_ps[:, b, :],
                                 m1[ci][:, b * 128:(b + 1) * 128],
                                 wm2b[:, ci, :], start=(ci == 0), stop=False)
            nc.tensor.matmul(out_ps[:, b, :], ones_row[:],
                             brows[:, R_BM2 * 128:(R_BM2 + 1) * 128], start=False, stop=True)
        out_sb = mp.tile([128, 4, 128], f32, name="out_sb")
        nc.vector.tensor_tensor(out=out_sb[:], in0=out_ps[:], in1=xt_sb[:],
                                op=OP.add)
        dump = os.environ.get("KERN_DUMP")
        if dump in ("agg", "num", "rinv"):
            src_ap = {"agg": agg[:], "num": num_ps[:].rearrange("p a b -> p (a b)"),
                      "rinv": rinv[:]}[dump]
            agg_f = mp.tile([128, 4, 128], f32, name="agg_f")
            nc.vector.tensor_copy(agg_f[:], src_ap.rearrange("p (a b) -> p a b", a=4))
            nc.sync.dma_start(
                io["out"][:][n0:n0 + 512, :].rearrange("(a p) d -> p a d", p=128),
                agg_f[:])
        else:
            nc.sync.dma_start(
                io["out"][:][n0:n0 + 512, :].rearrange("(a p) d -> p a d", p=128),
                out_sb[:])


# ------------------------------------------------------------------- runner
_CACHE = {}


def _build():
    nc = bacc.Bacc("TRN2", num_devices=NCORES)
    io = {}
    io["xt"] = nc.dram_tensor("xt", [SHARD, D], dt.float32, kind="ExternalInput").ap()
    io["xsrc"] = nc.dram_tensor("xsrc", [SRC_PAD, D], dt.float32, kind="ExternalInput").ap()
    io["maskc"] = nc.dram_tensor("maskc", [SRC_PAD, 1], dt.float32, kind="ExternalInput").ap()
    for name, shape in [("Wq", [D, D]), ("Wk", [D, D]), ("Wv", [D, D]),
                        ("Wih", [D, D]), ("Whh", [D, D]), ("Wo", [D, D]),
                        ("Wm1", [D, 4 * D]), ("Wm2", [4 * D, D])]:
        io[name] = nc.dram_tensor(name, shape, dt.float32, kind="ExternalInput").ap()
    io["brows"] = nc.dram_tensor("brows", [1, 16 * 128], dt.bfloat16, kind="ExternalInput").ap()
    io["bcols"] = nc.dram_tensor("bcols", [128, 8], dt.float32, kind="ExternalInput").ap()
    io["iota_bf"] = nc.dram_tensor("iota_bf", [128, 128], dt.bfloat16, kind="ExternalInput").ap()
    io["id_bf"] = nc.dram_tensor("id_bf", [128, 128], dt.bfloat16, kind="ExternalInput").ap()
    io["e8"] = nc.dram_tensor("e8", [8, 128], dt.float32, kind="ExternalInput").ap()
    io["idx_dst"] = nc.dram_tensor("idx_dst", [128, SLOTS // 16], dt.int16, kind="ExternalInput").ap()
    io["idx_kv"] = nc.dram_tensor("idx_kv", [128, SLOTS // 16], dt.int16, kind="ExternalInput").ap()
    io["rloc"] = nc.dram_tensor("rloc", [128, T], dt.float32, kind="ExternalInput").ap()
    io["q_bf"] = nc.dram_tensor("q_bf", [SHARD, D], dt.bfloat16, kind="Internal").ap()
    io["kv_bf"] = nc.dram_tensor("kv_bf", [KV_ROWS, 256], dt.bfloat16, kind="Internal").ap()
    io["xb_bf"] = nc.dram_tensor("xb_bf", [SHARD, D], dt.bfloat16, kind="Internal").ap()
    io["out"] = nc.dram_tensor("out", [SHARD, D], dt.float32, kind="ExternalOutput").ap()

    with tile.TileContext(nc) as tc:
        _program(tc, io)
    nc.compile()
    return nc


def kernel(**inputs):
    per_core, mask = _host_prep(inputs)

    if "nc" not in _CACHE:
        _CACHE["nc"] = _build()
    nc = _CACHE["nc"]

    xsrc = np.zeros((SRC_PAD, D), FP)
    xsrc[:N_SRC] = np.asarray(inputs["x_source"], FP)
    maskc = np.zeros((SRC_PAD, 1), FP)
    maskc[:N_SRC, 0] = mask.astype(FP)

    brows = np.zeros((1, 16 * 128), BF)
    for i, nm in enumerate(["bq", "bk", "bv", "bo", "bm2", "g3", "be3",
                            "g1", "be1"]):
        brows[0, i * 128:(i + 1) * 128] = np.asarray(inputs[nm]).astype(BF)
    bcols = np.zeros((128, 8), FP)
    bcols[:, 0] = np.asarray(inputs["bih"], FP)
    bcols[:, 1] = np.asarray(inputs["bhh"], FP)
    bcols[:, 2:6] = np.asarray(inputs["bm1"], FP).reshape(4, 128).T
    iota_bf = np.tile(np.arange(128, dtype=FP), (128, 1)).astype(BF)
    id_bf = np.eye(128, dtype=FP).astype(BF)
    e8 = np.zeros((8, 128), FP)
    for h in range(H):
        e8[h, h * DH:(h + 1) * DH] = 1.0

    shared = {
        "xsrc": xsrc, "maskc": maskc, "brows": brows, "bcols": bcols,
        "iota_bf": iota_bf, "id_bf": id_bf, "e8": e8,
    }
    for nm in ["Wq", "Wk", "Wv", "Wih", "Whh", "Wo", "Wm1", "Wm2"]:
        shared[nm] = np.asarray(inputs[nm], FP)

    x_target = np.asarray(inputs["x_target"], FP)
    in_maps = []
    for c in range(NCORES):
        m = dict(shared)
        m["xt"] = np.ascontiguousarray(x_target[c * SHARD:(c + 1) * SHARD])
        m["idx_dst"] = per_core[c]["idx_dst"]
        m["idx_kv"] = per_core[c]["idx_kv"]
        m["rloc"] = per_core[c]["rloc"]
        in_maps.append(m)

    trace = os.environ.get("KERN_TRACE") == "1"
    res = bass_utils.run_bass_kernel_spmd(nc, in_maps, core_ids=list(range(NCORES)),
                                          trace=trace)
    _CACHE["last"] = (nc, in_maps, res)
    out = np.concatenate([r["out"] for r in res.results], axis=0)
    return out.astype(np.float32)


if __name__ == "__main__":
    import importlib.util
    spec = importlib.util.spec_from_file_location("reference", "/root/problem/reference.py")
    ref = importlib.util.module_from_spec(spec)
    spec.loader.exec_module(ref)
    inputs = {k: np.asarray(v) for k, v in ref.setup_inputs().items()}
    out = kernel(**inputs)
    exp = np.asarray(ref.reference(**inputs))
    err = np.abs(out - exp)
    print("absmax:", err.max(), "rel:", err.max() / np.abs(exp).max())
